# revision 11
# baseline (speedup 1.0000x reference)
"""Trainium2 Bass kernel for DiffusionGraphConv (gnn message passing).

Math (reference):
    x0 = concat(inputs, state) per node            # [B, N, F=128] -> [N, F*B]
    x1 = S @ x0                                    # COO spmm, rows sorted
    x2 = 2 * S @ x1 - x0
    out = concat_k(xk) @ weight + bias             # [B*N, F*3] @ [384, 128]

Folded form used here (S and W commute):
    out = X0 @ (W0 - W2) + X1 @ W1 + Y2 @ (2*W2),  Y2 = S @ X1, X1 = S @ X0
    with W_k = weight[k::3, :]

Sharding: data-parallel over batch. 8 cores x 2 batches each; each core runs
the full graph spmm over its local width W = 2*128 = 256.

Device algorithm per core:
  - spmm: edges are row-sorted, grouped into 128-row blocks, padded to
    128-slot chunks. Per block: one dma_gather pulls z[col] rows (bf16,
    512B each) from a DRAM table into SBUF at [slot-partition, W]. Per chunk:
    a one-hot selection matrix M[slot, r] = v_slot * (rowloc[slot] == r) is
    built by a single fused DVE tensor_scalar op (iota == rowloc) * v, and
    TensorE accumulates M.T @ G into PSUM [128 rows, W].
  - x1 is written back as the bf16 DRAM gather table for spmm #2.
  - XBAR transpose DMAs produce feature-major bf16 copies of x1/y2 for the
    output projection; x0 arrives feature-major fp32 from the host.
  - Output projection: PSUM accumulation of 3 matmuls (Wa fp32, Wb/Wc bf16
    stationary), bias added during the PSUM->SBUF copy on ScalarE.
"""

import os
import sys

for _p in ("/root/.axon_site/_ro/trn_rl_repo", "/opt/trn_rl_repo"):
    if os.path.isdir(_p) and _p not in sys.path:
        sys.path.append(_p)

import numpy as np
import ml_dtypes

BF16 = ml_dtypes.bfloat16

N_CORES = 8
B = 16
N = 10000
IN_DIM = 64
F = 128          # input + hidden features per node per batch
BL = B // N_CORES  # local batches per core = 2
W = BL * F       # local spmm width = 256
OUT = 128
RB = 128         # row-block size
CH3 = 512        # stage-3 node chunk


class GraphMeta:
    """Host-side prep of the COO graph into chunked, padded device arrays."""

    def __init__(self, rows, cols, vals, n=N):
        rows = np.asarray(rows, dtype=np.int64)
        cols = np.asarray(cols, dtype=np.int64)
        vals = np.asarray(vals, dtype=np.float32)
        order = np.argsort(rows, kind="stable")
        if not np.all(rows[:-1] <= rows[1:]):
            rows, cols, vals = rows[order], cols[order], vals[order]
        self.n = n
        self.nblocks = (n + RB - 1) // RB
        bounds = np.searchsorted(rows, np.arange(self.nblocks + 1) * RB)
        counts = bounds[1:] - bounds[:-1]
        self.nch = np.maximum(1, (counts + 127) // 128).astype(np.int64)
        self.off = np.concatenate([[0], np.cumsum(self.nch)])
        self.total_nch = int(self.off[-1])
        self.max_nch = int(self.nch.max())

        tc = self.total_nch
        gidx = np.zeros((128, tc * 8), dtype=np.int16)
        rowloc = np.zeros((128, tc), dtype=np.float32)
        valarr = np.zeros((128, tc), dtype=np.float32)
        for b in range(self.nblocks):
            e0, e1 = bounds[b], bounds[b + 1]
            cnt = e1 - e0
            padded = int(self.nch[b]) * 128
            cp = np.zeros(padded, dtype=np.int64)
            rp = np.zeros(padded, dtype=np.float32)
            vp = np.zeros(padded, dtype=np.float32)
            cp[:cnt] = cols[e0:e1]
            rp[:cnt] = (rows[e0:e1] - b * RB).astype(np.float32)
            vp[:cnt] = vals[e0:e1]
            ii = np.arange(padded)
            for q in range(8):
                gidx[16 * q + ii % 16, int(self.off[b]) * 8 + ii // 16] = (
                    cp.astype(np.int16)
                )
            rowloc[ii % 128, int(self.off[b]) + ii // 128] = rp
            valarr[ii % 128, int(self.off[b]) + ii // 128] = vp
        self.gidx = gidx
        self.rowloc = rowloc
        self.vals = valarr


def build_nc(g, n=N, w=W, out_dim=OUT, n_cores=N_CORES, reps=1):
    """Build + compile the SPMD Bass program (identical across cores)."""
    from contextlib import ExitStack

    import concourse.bacc as bacc
    import concourse.mybir as mybir
    import concourse.tile as tile
    from concourse.library_config import mlp
    from concourse.masks import make_identity

    f32 = mybir.dt.float32
    bf16 = mybir.dt.bfloat16
    i16 = mybir.dt.int16
    AO = mybir.AluOpType

    nb = g.nblocks
    tc_n = g.total_nch

    nc = bacc.Bacc(
        "TRN2", target_bir_lowering=False, debug=False, num_devices=n_cores
    )

    x0t_d = nc.declare_dram_parameter("x0t", [BL, F, n], f32, isOutput=False)
    z0_d = nc.declare_dram_parameter("z0", [n, w], bf16, isOutput=False)
    gidx_d = nc.declare_dram_parameter("gidx", [128, tc_n * 8], i16, isOutput=False)
    rl_d = nc.declare_dram_parameter("rowloc", [128, tc_n], f32, isOutput=False)
    vv_d = nc.declare_dram_parameter("vals", [128, tc_n], f32, isOutput=False)
    iota_d = nc.declare_dram_parameter("iota", [128, 128], f32, isOutput=False)
    wa_d = nc.declare_dram_parameter("wa", [F, out_dim], f32, isOutput=False)
    wb_d = nc.declare_dram_parameter("wb", [F, out_dim], bf16, isOutput=False)
    wc_d = nc.declare_dram_parameter("wc", [F, out_dim], bf16, isOutput=False)
    bias_d = nc.declare_dram_parameter("bias", [out_dim, 1], f32, isOutput=False)
    out_d = nc.declare_dram_parameter("outp", [BL, out_dim, n], f32, isOutput=True)

    z1_d = nc.dram_tensor("z1", [n, w], bf16)

    with tile.TileContext(nc) as tc, ExitStack() as ctx:
        nc.gpsimd.load_library(mlp)
        tc.strict_bb_all_engine_barrier()
        const = ctx.enter_context(tc.tile_pool(name="const", bufs=1))
        iota_sb = const.tile([128, 128], f32)
        nc.sync.dma_start(out=iota_sb[:], in_=iota_d[:])
        wa_sb = const.tile([F, out_dim], f32)
        nc.sync.dma_start(out=wa_sb[:], in_=wa_d[:])
        wb_sb = const.tile([F, out_dim], bf16)
        nc.sync.dma_start(out=wb_sb[:], in_=wb_d[:])
        wc_sb = const.tile([F, out_dim], bf16)
        nc.sync.dma_start(out=wc_sb[:], in_=wc_d[:])
        bias_sb = const.tile([out_dim, 1], f32)
        nc.sync.dma_start(out=bias_sb[:], in_=bias_d[:])
        gidx_sb = const.tile([128, tc_n * 8], i16)
        nc.sync.dma_start(out=gidx_sb[:], in_=gidx_d[:])
        rl_sb = const.tile([128, tc_n], f32)
        nc.sync.dma_start(out=rl_sb[:], in_=rl_d[:])
        vv_sb = const.tile([128, tc_n], f32)
        nc.sync.dma_start(out=vv_sb[:], in_=vv_d[:])
        ident_sb = const.tile([128, 128], f32)
        make_identity(nc, ident_sb[:])

        gpool = ctx.enter_context(tc.tile_pool(name="g", bufs=3))
        mpool = ctx.enter_context(tc.tile_pool(name="m", bufs=8))
        pspool = ctx.enter_context(tc.tile_pool(name="ps", bufs=3, space="PSUM"))
        pstpool = ctx.enter_context(tc.tile_pool(name="pst", bufs=2, space="PSUM"))
        xopool = ctx.enter_context(tc.tile_pool(name="xo", bufs=4))
        xt = ctx.enter_context(tc.tile_pool(name="xt", bufs=1))
        x1t = xt.tile([128, BL, n], bf16)
        y2t = xt.tile([128, BL, n], bf16)

        def spmm(src_d, dst_d, dst_t):
            for b in range(nb):
                nch = int(g.nch[b])
                off = int(g.off[b])
                G = gpool.tile([128, g.max_nch, w], bf16, tag="G")
                nc.gpsimd.dma_gather(
                    out_ap=G[:, :nch, :],
                    in_ap=src_d[:, :],
                    idxs_ap=gidx_sb[:, off * 8 : (off + nch) * 8],
                    num_idxs=nch * 128,
                    num_idxs_reg=nch * 128,
                    elem_size=w,
                    single_packet=False,
                )
                ps = pspool.tile([128, w], f32, tag="ps")
                for c in range(nch):
                    M = mpool.tile([128, 128], bf16, tag="M")
                    nc.vector.tensor_scalar(
                        out=M[:],
                        in0=iota_sb[:],
                        scalar1=rl_sb[:, off + c : off + c + 1],
                        scalar2=vv_sb[:, off + c : off + c + 1],
                        op0=AO.is_equal,
                        op1=AO.mult,
                    )
                    nc.tensor.matmul(
                        out=ps[:],
                        lhsT=M[:],
                        rhs=G[:, c, :],
                        start=(c == 0),
                        stop=(c == nch - 1),
                    )
                xo = xopool.tile([128, w], f32, tag="xo")
                nc.vector.tensor_copy(out=xo[:], in_=ps[:])
                r0 = b * RB
                r1 = min(n, r0 + RB)
                if dst_d is not None:
                    nc.gpsimd.dma_start(out=dst_d[r0:r1, :], in_=xo[: r1 - r0, :])
                for h in range(BL):
                    pst = pstpool.tile([128, 128], f32, tag="pst")
                    nc.tensor.transpose(
                        out=pst[:],
                        in_=xo[:, h * 128 : (h + 1) * 128],
                        identity=ident_sb[:],
                    )
                    nc.vector.tensor_copy(
                        out=dst_t[:, h, r0:r1], in_=pst[:, : r1 - r0]
                    )

        s3 = ctx.enter_context(tc.tile_pool(name="s3", bufs=3))
        ps3pool = ctx.enter_context(tc.tile_pool(name="ps3", bufs=2, space="PSUM"))

        def stage3():

            for bl in range(BL):
                for t0 in range(0, n, CH3):
                    cn = min(CH3, n - t0)
                    x0c = s3.tile([128, CH3], f32, tag="x0c")
                    nc.sync.dma_start(out=x0c[:, :cn], in_=x0t_d[bl, :, t0 : t0 + cn])
                    ps3 = ps3pool.tile([128, CH3], f32, tag="ps3")
                    nc.tensor.matmul(
                        out=ps3[:, :cn], lhsT=wa_sb[:], rhs=x0c[:, :cn],
                        start=True, stop=False,
                    )
                    nc.tensor.matmul(
                        out=ps3[:, :cn], lhsT=wb_sb[:], rhs=x1t[:, bl, t0 : t0 + cn],
                        start=False, stop=False,
                    )
                    nc.tensor.matmul(
                        out=ps3[:, :cn], lhsT=wc_sb[:], rhs=y2t[:, bl, t0 : t0 + cn],
                        start=False, stop=True,
                    )
                    oc = s3.tile([128, CH3], f32, tag="oc")
                    nc.scalar.activation(
                        oc[:, :cn],
                        ps3[:, :cn],
                        mybir.ActivationFunctionType.Identity,
                        bias=bias_sb[:],
                    )
                    nc.sync.dma_start(out=out_d[bl, :, t0 : t0 + cn], in_=oc[:, :cn])

        for _rep in range(reps):
            if _rep:
                tc.strict_bb_all_engine_barrier()
            spmm(z0_d, z1_d, x1t)
            tc.strict_bb_all_engine_barrier()
            spmm(z1_d, None, y2t)
            tc.strict_bb_all_engine_barrier()
            stage3()

    nc.compile()
    return nc


def _host_prep(inputs, state, support_rows, support_cols, support_vals, weight, biases):
    inputs = np.asarray(inputs, dtype=np.float32)
    state = np.asarray(state, dtype=np.float32)
    weight = np.asarray(weight, dtype=np.float32)
    biases = np.asarray(biases, dtype=np.float32)

    g = GraphMeta(support_rows, support_cols, support_vals)

    # per-batch [N, F] node features (inputs | state)
    xb = np.concatenate(
        [inputs.reshape(B, N, IN_DIM), state.reshape(B, N, IN_DIM)], axis=2
    )  # [B, N, 128]

    w0, w1, w2 = weight[0::3], weight[1::3], weight[2::3]
    wa = (w0 - w2).astype(np.float32)
    wb = w1.astype(BF16)
    wc = (2.0 * w2).astype(BF16)
    bias = biases.reshape(OUT, 1).astype(np.float32)
    iota = np.ascontiguousarray(np.broadcast_to(np.arange(128, dtype=np.float32), (128, 128)))

    in_maps = []
    for k in range(N_CORES):
        bs = [BL * k + j for j in range(BL)]
        z0 = np.concatenate([xb[b] for b in bs], axis=1).astype(BF16)  # [N, 256]
        x0t = np.stack([xb[b].T for b in bs]).astype(np.float32)  # [2, 128, N]
        in_maps.append(
            {
                "x0t": np.ascontiguousarray(x0t),
                "z0": np.ascontiguousarray(z0),
                "gidx": g.gidx,
                "rowloc": g.rowloc,
                "vals": g.vals,
                "iota": iota,
                "wa": wa,
                "wb": wb,
                "wc": wc,
                "bias": bias,
            }
        )
    return g, in_maps


def run_full(
    inputs,
    state,
    support_rows,
    support_cols,
    support_vals,
    weight,
    biases,
    output_size=OUT,
    trace=False,
    tmpdir=None,
):
    """Run the kernel; returns (output, BassKernelResults)."""
    g, in_maps = _host_prep(
        inputs, state, support_rows, support_cols, support_vals, weight, biases
    )
    nc = build_nc(g)

    from concourse.bass_utils import run_bass_kernel_spmd

    res = run_bass_kernel_spmd(
        nc, in_maps, core_ids=list(range(N_CORES)), trace=trace, tmpdir=tmpdir
    )

    out = np.empty((B, N * OUT), dtype=np.float32)
    for k in range(N_CORES):
        r = np.asarray(res.results[k]["outp"])  # [BL, OUT, N]
        for j in range(BL):
            out[BL * k + j] = r[j].T.reshape(-1)
    return out, res


def kernel(
    inputs,
    state,
    support_rows,
    support_cols,
    support_vals,
    weight,
    biases,
    output_size,
):
    out, _ = run_full(
        inputs, state, support_rows, support_cols, support_vals, weight, biases
    )
    return out


# revision 12
# speedup vs baseline: 1.2131x; 1.2131x over previous
"""Trainium2 Bass kernel for DiffusionGraphConv (gnn message passing).

Math (reference):
    x0 = concat(inputs, state) per node            # [B, N, F=128] -> [N, F*B]
    x1 = S @ x0                                    # COO spmm, rows sorted
    x2 = 2 * S @ x1 - x0
    out = concat_k(xk) @ weight + bias             # [B*N, F*3] @ [384, 128]

Folded form used here (S and W commute):
    out = X0 @ (W0 - W2) + X1 @ W1 + Y2 @ (2*W2),  Y2 = S @ X1, X1 = S @ X0
    with W_k = weight[k::3, :]

Sharding: data-parallel over batch. 8 cores x 2 batches each; each core runs
the full graph spmm over its local width W = 2*128 = 256.

Device algorithm per core:
  - spmm: edges are row-sorted, grouped into 128-row blocks, padded to
    128-slot chunks. Per block: one dma_gather pulls z[col] rows (bf16,
    512B each) from a DRAM table into SBUF at [slot-partition, W]. Per chunk:
    a one-hot selection matrix M[slot, r] = v_slot * (rowloc[slot] == r) is
    built by a single fused DVE tensor_scalar op (iota == rowloc) * v, and
    TensorE accumulates M.T @ G into PSUM [128 rows, W].
  - x1 is written back as the bf16 DRAM gather table for spmm #2.
  - XBAR transpose DMAs produce feature-major bf16 copies of x1/y2 for the
    output projection; x0 arrives feature-major fp32 from the host.
  - Output projection: PSUM accumulation of 3 matmuls (Wa fp32, Wb/Wc bf16
    stationary), bias added during the PSUM->SBUF copy on ScalarE.
"""

import os
import sys

for _p in ("/root/.axon_site/_ro/trn_rl_repo", "/opt/trn_rl_repo"):
    if os.path.isdir(_p) and _p not in sys.path:
        sys.path.append(_p)

import numpy as np
import ml_dtypes

BF16 = ml_dtypes.bfloat16

N_CORES = 8
B = 16
N = 10000
IN_DIM = 64
F = 128          # input + hidden features per node per batch
BL = B // N_CORES  # local batches per core = 2
W = BL * F       # local spmm width = 256
OUT = 128
RB = 128         # row-block size
CH3 = 512        # stage-3 node chunk


class GraphMeta:
    """Host-side prep of the COO graph into chunked, padded device arrays."""

    def __init__(self, rows, cols, vals, n=N):
        rows = np.asarray(rows, dtype=np.int64)
        cols = np.asarray(cols, dtype=np.int64)
        vals = np.asarray(vals, dtype=np.float32)
        order = np.argsort(rows, kind="stable")
        if not np.all(rows[:-1] <= rows[1:]):
            rows, cols, vals = rows[order], cols[order], vals[order]
        self.n = n
        self.nblocks = (n + RB - 1) // RB
        bounds = np.searchsorted(rows, np.arange(self.nblocks + 1) * RB)
        counts = bounds[1:] - bounds[:-1]
        self.nch = np.maximum(1, (counts + 127) // 128).astype(np.int64)
        self.off = np.concatenate([[0], np.cumsum(self.nch)])
        self.total_nch = int(self.off[-1])
        self.max_nch = int(self.nch.max())

        tc = self.total_nch
        gidx = np.zeros((128, tc * 8), dtype=np.int16)
        rowloc = np.zeros((128, tc), dtype=np.float32)
        valarr = np.zeros((128, tc), dtype=np.float32)
        for b in range(self.nblocks):
            e0, e1 = bounds[b], bounds[b + 1]
            cnt = e1 - e0
            padded = int(self.nch[b]) * 128
            cp = np.zeros(padded, dtype=np.int64)
            rp = np.zeros(padded, dtype=np.float32)
            vp = np.zeros(padded, dtype=np.float32)
            cp[:cnt] = cols[e0:e1]
            rp[:cnt] = (rows[e0:e1] - b * RB).astype(np.float32)
            vp[:cnt] = vals[e0:e1]
            ii = np.arange(padded)
            for q in range(8):
                gidx[16 * q + ii % 16, int(self.off[b]) * 8 + ii // 16] = (
                    cp.astype(np.int16)
                )
            rowloc[ii % 128, int(self.off[b]) + ii // 128] = rp
            valarr[ii % 128, int(self.off[b]) + ii // 128] = vp
        self.gidx = gidx
        self.rowloc = rowloc
        self.vals = valarr


def build_nc(g, n=N, w=W, out_dim=OUT, n_cores=N_CORES, reps=1):
    """Build + compile the SPMD Bass program (identical across cores)."""
    from contextlib import ExitStack

    import concourse.bacc as bacc
    import concourse.mybir as mybir
    import concourse.tile as tile
    from concourse.library_config import mlp
    from concourse.masks import make_identity

    f32 = mybir.dt.float32
    bf16 = mybir.dt.bfloat16
    i16 = mybir.dt.int16
    AO = mybir.AluOpType

    nb = g.nblocks
    tc_n = g.total_nch

    nc = bacc.Bacc(
        "TRN2", target_bir_lowering=False, debug=False, num_devices=n_cores
    )

    x0t_d = nc.declare_dram_parameter("x0t", [BL, F, n], f32, isOutput=False)
    z0_d = nc.declare_dram_parameter("z0", [n, w], bf16, isOutput=False)
    gidx_d = nc.declare_dram_parameter("gidx", [128, tc_n * 8], i16, isOutput=False)
    rl_d = nc.declare_dram_parameter("rowloc", [128, tc_n], f32, isOutput=False)
    vv_d = nc.declare_dram_parameter("vals", [128, tc_n], f32, isOutput=False)
    iota_d = nc.declare_dram_parameter("iota", [128, 128], f32, isOutput=False)
    wa_d = nc.declare_dram_parameter("wa", [F, out_dim], f32, isOutput=False)
    wb_d = nc.declare_dram_parameter("wb", [F, out_dim], bf16, isOutput=False)
    wc_d = nc.declare_dram_parameter("wc", [F, out_dim], bf16, isOutput=False)
    bias_d = nc.declare_dram_parameter("bias", [out_dim, 1], f32, isOutput=False)
    out_d = nc.declare_dram_parameter("outp", [BL, out_dim, n], f32, isOutput=True)

    z1_d = nc.dram_tensor("z1", [n, w], bf16)

    with tile.TileContext(nc) as tc, ExitStack() as ctx:
        nc.gpsimd.load_library(mlp)
        tc.strict_bb_all_engine_barrier()
        const = ctx.enter_context(tc.tile_pool(name="const", bufs=1))
        iota_sb = const.tile([128, 128], f32)
        nc.sync.dma_start(out=iota_sb[:], in_=iota_d[:])
        wa_sb = const.tile([F, out_dim], f32)
        nc.sync.dma_start(out=wa_sb[:], in_=wa_d[:])
        wb_sb = const.tile([F, out_dim], bf16)
        nc.sync.dma_start(out=wb_sb[:], in_=wb_d[:])
        wc_sb = const.tile([F, out_dim], bf16)
        nc.sync.dma_start(out=wc_sb[:], in_=wc_d[:])
        bias_sb = const.tile([out_dim, 1], f32)
        nc.sync.dma_start(out=bias_sb[:], in_=bias_d[:])
        gidx_sb = const.tile([128, tc_n * 8], i16)
        nc.sync.dma_start(out=gidx_sb[:], in_=gidx_d[:])
        rl_sb = const.tile([128, tc_n], f32)
        nc.sync.dma_start(out=rl_sb[:], in_=rl_d[:])
        vv_sb = const.tile([128, tc_n], f32)
        nc.sync.dma_start(out=vv_sb[:], in_=vv_d[:])
        ident_sb = const.tile([128, 128], f32)
        make_identity(nc, ident_sb[:])

        gpool = ctx.enter_context(tc.tile_pool(name="g", bufs=3))
        mpool = ctx.enter_context(tc.tile_pool(name="m", bufs=8))
        pspool = ctx.enter_context(tc.tile_pool(name="ps", bufs=3, space="PSUM"))
        pstpool = ctx.enter_context(tc.tile_pool(name="pst", bufs=2, space="PSUM"))
        xopool = ctx.enter_context(tc.tile_pool(name="xo", bufs=4))
        xt = ctx.enter_context(tc.tile_pool(name="xt", bufs=1))
        x1t = xt.tile([128, BL, n], bf16)
        y2t = xt.tile([128, BL, n], bf16)

        def spmm(src_d, dst_d, dst_t):
            for b in range(nb):
                nch = int(g.nch[b])
                off = int(g.off[b])
                G = gpool.tile([128, g.max_nch, w], bf16, tag="G")
                GSP = 2  # chunks per gather: 256 idxs, single-packet fast path
                for c0 in range(0, nch, GSP):
                    cw = min(GSP, nch - c0)
                    nc.gpsimd.dma_gather(
                        out_ap=G[:, c0 : c0 + cw, :],
                        in_ap=src_d[:, :],
                        idxs_ap=gidx_sb[:, (off + c0) * 8 : (off + c0 + cw) * 8],
                        num_idxs=cw * 128,
                        num_idxs_reg=cw * 128,
                        elem_size=w,
                        single_packet=True,
                    )
                ps = pspool.tile([128, w], f32, tag="ps")
                for c in range(nch):
                    M = mpool.tile([128, 128], bf16, tag="M")
                    nc.vector.tensor_scalar(
                        out=M[:],
                        in0=iota_sb[:],
                        scalar1=rl_sb[:, off + c : off + c + 1],
                        scalar2=vv_sb[:, off + c : off + c + 1],
                        op0=AO.is_equal,
                        op1=AO.mult,
                    )
                    nc.tensor.matmul(
                        out=ps[:],
                        lhsT=M[:],
                        rhs=G[:, c, :],
                        start=(c == 0),
                        stop=(c == nch - 1),
                    )
                xo = xopool.tile([128, w], f32, tag="xo")
                nc.vector.tensor_copy(out=xo[:], in_=ps[:])
                r0 = b * RB
                r1 = min(n, r0 + RB)
                if dst_d is not None:
                    nc.gpsimd.dma_start(out=dst_d[r0:r1, :], in_=xo[: r1 - r0, :])
                for h in range(BL):
                    pst = pstpool.tile([128, 128], f32, tag="pst")
                    nc.tensor.transpose(
                        out=pst[:],
                        in_=xo[:, h * 128 : (h + 1) * 128],
                        identity=ident_sb[:],
                    )
                    nc.vector.tensor_copy(
                        out=dst_t[:, h, r0:r1], in_=pst[:, : r1 - r0]
                    )

        s3 = ctx.enter_context(tc.tile_pool(name="s3", bufs=3))
        ps3pool = ctx.enter_context(tc.tile_pool(name="ps3", bufs=2, space="PSUM"))

        def stage3():

            for bl in range(BL):
                for t0 in range(0, n, CH3):
                    cn = min(CH3, n - t0)
                    x0c = s3.tile([128, CH3], f32, tag="x0c")
                    nc.sync.dma_start(out=x0c[:, :cn], in_=x0t_d[bl, :, t0 : t0 + cn])
                    ps3 = ps3pool.tile([128, CH3], f32, tag="ps3")
                    nc.tensor.matmul(
                        out=ps3[:, :cn], lhsT=wa_sb[:], rhs=x0c[:, :cn],
                        start=True, stop=False,
                    )
                    nc.tensor.matmul(
                        out=ps3[:, :cn], lhsT=wb_sb[:], rhs=x1t[:, bl, t0 : t0 + cn],
                        start=False, stop=False,
                    )
                    nc.tensor.matmul(
                        out=ps3[:, :cn], lhsT=wc_sb[:], rhs=y2t[:, bl, t0 : t0 + cn],
                        start=False, stop=True,
                    )
                    oc = s3.tile([128, CH3], f32, tag="oc")
                    nc.scalar.activation(
                        oc[:, :cn],
                        ps3[:, :cn],
                        mybir.ActivationFunctionType.Identity,
                        bias=bias_sb[:],
                    )
                    nc.sync.dma_start(out=out_d[bl, :, t0 : t0 + cn], in_=oc[:, :cn])

        for _rep in range(reps):
            if _rep:
                tc.strict_bb_all_engine_barrier()
            spmm(z0_d, z1_d, x1t)
            tc.strict_bb_all_engine_barrier()
            spmm(z1_d, None, y2t)
            tc.strict_bb_all_engine_barrier()
            stage3()

    nc.compile()
    return nc


def _host_prep(inputs, state, support_rows, support_cols, support_vals, weight, biases):
    inputs = np.asarray(inputs, dtype=np.float32)
    state = np.asarray(state, dtype=np.float32)
    weight = np.asarray(weight, dtype=np.float32)
    biases = np.asarray(biases, dtype=np.float32)

    g = GraphMeta(support_rows, support_cols, support_vals)

    # per-batch [N, F] node features (inputs | state)
    xb = np.concatenate(
        [inputs.reshape(B, N, IN_DIM), state.reshape(B, N, IN_DIM)], axis=2
    )  # [B, N, 128]

    w0, w1, w2 = weight[0::3], weight[1::3], weight[2::3]
    wa = (w0 - w2).astype(np.float32)
    wb = w1.astype(BF16)
    wc = (2.0 * w2).astype(BF16)
    bias = biases.reshape(OUT, 1).astype(np.float32)
    iota = np.ascontiguousarray(np.broadcast_to(np.arange(128, dtype=np.float32), (128, 128)))

    in_maps = []
    for k in range(N_CORES):
        bs = [BL * k + j for j in range(BL)]
        z0 = np.concatenate([xb[b] for b in bs], axis=1).astype(BF16)  # [N, 256]
        x0t = np.stack([xb[b].T for b in bs]).astype(np.float32)  # [2, 128, N]
        in_maps.append(
            {
                "x0t": np.ascontiguousarray(x0t),
                "z0": np.ascontiguousarray(z0),
                "gidx": g.gidx,
                "rowloc": g.rowloc,
                "vals": g.vals,
                "iota": iota,
                "wa": wa,
                "wb": wb,
                "wc": wc,
                "bias": bias,
            }
        )
    return g, in_maps


def run_full(
    inputs,
    state,
    support_rows,
    support_cols,
    support_vals,
    weight,
    biases,
    output_size=OUT,
    trace=False,
    tmpdir=None,
):
    """Run the kernel; returns (output, BassKernelResults)."""
    g, in_maps = _host_prep(
        inputs, state, support_rows, support_cols, support_vals, weight, biases
    )
    nc = build_nc(g)

    from concourse.bass_utils import run_bass_kernel_spmd

    res = run_bass_kernel_spmd(
        nc, in_maps, core_ids=list(range(N_CORES)), trace=trace, tmpdir=tmpdir
    )

    out = np.empty((B, N * OUT), dtype=np.float32)
    for k in range(N_CORES):
        r = np.asarray(res.results[k]["outp"])  # [BL, OUT, N]
        for j in range(BL):
            out[BL * k + j] = r[j].T.reshape(-1)
    return out, res


def kernel(
    inputs,
    state,
    support_rows,
    support_cols,
    support_vals,
    weight,
    biases,
    output_size,
):
    out, _ = run_full(
        inputs, state, support_rows, support_cols, support_vals, weight, biases
    )
    return out
